# revision 71
# baseline (speedup 1.0000x reference)
"""Trainium2 Bass kernel for per-head-projection MultiHeadAttention (v3).

Contract: kernel(**inputs) takes the FULL unsharded inputs (as produced by
reference.setup_inputs()) and returns the FULL [B, S, D] output.

Sharding (tensor-parallel over heads x data-parallel over batch):
  - 8 cores; cores 0-3 handle batch 0, cores 4-7 handle batch 1.
  - Each core owns 4 heads (two "head pairs"). It computes Q/K/V projections
    for those heads, causal attention, and a partial output projection
    (ctx @ Wo rows for its heads). The host sums the 4 partials per batch
    (the output linear is linear over head blocks) and adds bo.

v3 (vs v2): bf16 activations/weights on device (fp32 PSUM accumulation);
host pre-packs DRAM tensors in SBUF layout so every load is one large DMA
(DMA-issue overhead: 565ns SP.SEQ + 625ns shared HWDGE + 900ns completion
semaphore each); the softmax denominator is broadcast across partitions
with gpsimd partition_broadcast instead of a DRAM bounce. Work is spread
across engines: exp on ACT, PSUM->SBUF moves/reciprocals on DVE,
diag-mask muls + broadcasts on gpsimd (which cannot access PSUM).

The attention inner loop is Activation(exp)-paced, so all projection and
output-projection matmuls are emitted as "pieces" interleaved into the
attention kc-loops (evenly paced across each phase, emitted BEFORE the
exp-dependent PV so the PE's 4-deep stall window never blocks them);
outproj pieces are deferred to the last, longest attention phases.
Scores are computed transposed ([keys, queries]) so softmax needs no
transposes; the denominator comes from a ones-column in V-natural (built
with PE transposes; the DMA-XBAR transpose corrupts data on HW); exp runs
fused over both heads ([128,1024] PSUM) with the 1/sqrt(DH) scale folded
in. Sharded tensor-parallel over heads x data-parallel over batch as v2.
"""

import sys

sys.path.insert(0, "/opt/trn_rl_repo")

import numpy as np

B, S, D, H = 2, 2048, 1024, 16
DH = D // H            # 64
NCORES = 8
HPC = H * B // NCORES  # 4 heads per core
NPAIR = HPC // 2       # 2 head pairs per core
SG = 512               # s-group / query-group size
NSG = S // SG          # 4
NKT = S // 128         # 16 key tiles
NDC = D // 128         # 8 contraction chunks

_BUILD_CACHE = {}


def _build(causal: bool):
    """Build + compile the per-core Bass program. Cached per causal flag."""
    import concourse.bass as bass
    import concourse.bacc as bacc
    import concourse.tile as tile
    from concourse import mybir

    f32 = mybir.dt.float32
    f32r = mybir.dt.float32r
    bf16 = mybir.dt.bfloat16
    EXP = mybir.ActivationFunctionType.Exp

    nc = bacc.Bacc("TRN2", target_bir_lowering=False, debug=False)

    # DRAM tensors, already in SBUF layout (host packs):
    #   x*  [128, NDC, S]        bf16: x*[p, c, s] = x[s, c*128+p] (x^T rows)
    #   w*  [128, NPAIR, NDC, 128] bf16 per-pair weight stacks
    #   wo  [128, NPAIR, D]      bf16
    #   bqkv [128, 3, NPAIR]     f32 biases
    #   mk  [128, 128]           bf16 lower-triangular ones
    #   on2 [2, 128]             f32r ones2[0,0:64]=1, ones2[1,64:128]=1
    xq = nc.dram_tensor("xq", [128, NDC, S], bf16, kind="ExternalInput").ap()
    xk = nc.dram_tensor("xk", [128, NDC, S], bf16, kind="ExternalInput").ap()
    xv = nc.dram_tensor("xv", [128, NDC, S], bf16, kind="ExternalInput").ap()
    wq = nc.dram_tensor("wq", [128, NPAIR, NDC, 128], bf16, kind="ExternalInput").ap()
    wk = nc.dram_tensor("wk", [128, NPAIR, NDC, 128], bf16, kind="ExternalInput").ap()
    wv = nc.dram_tensor("wv", [128, NPAIR, NDC, 128], bf16, kind="ExternalInput").ap()
    wo = nc.dram_tensor("wo", [128, NPAIR, D], bf16, kind="ExternalInput").ap()
    bqkv = nc.dram_tensor("bqkv", [128, 3, NPAIR], f32, kind="ExternalInput").ap()
    mk = nc.dram_tensor("mk", [128, 2, 128], bf16, kind="ExternalInput").ap()
    idm = nc.dram_tensor("idm", [128, 64], bf16, kind="ExternalInput").ap()
    out = nc.dram_tensor("out", [S, D], bf16, kind="ExternalOutput").ap()
    import os

    DBG = bool(os.environ.get("KERNEL_DEBUG"))
    if DBG:
        dbg_qT = nc.dram_tensor("dbg_qT", [128, NPAIR, S], bf16, kind="ExternalOutput").ap()
        dbg_kT = nc.dram_tensor("dbg_kT", [128, NPAIR, S], bf16, kind="ExternalOutput").ap()
        dbg_vN = nc.dram_tensor(
            "dbg_vN", [128, HPC, NKT, 65], bf16, kind="ExternalOutput"
        ).ap()
        dbg_ctxn = nc.dram_tensor(
            "dbg_ctxn", [128, NPAIR, S], bf16, kind="ExternalOutput"
        ).ap()

    with tile.TileContext(nc) as tc:
        with (
            tc.tile_pool(name="persist", bufs=1) as persist,
            tc.tile_pool(name="vtgs", bufs=3) as vt_pool,
            tc.tile_pool(name="pts", bufs=8) as pt_pool,
            tc.tile_pool(name="invs", bufs=6) as st_pool,
            tc.tile_pool(name="outs", bufs=8) as out_pool,
            tc.tile_pool(name="psma", bufs=2, space="PSUM") as psA,
            tc.tile_pool(name="psmb", bufs=2, space="PSUM") as psB,
            tc.tile_pool(name="psmc", bufs=2, space="PSUM") as psC,
        ):
            # ---- consts (gpsimd/SWDGE queue; tiny, never transfer-critical)
            mask = persist.tile([128, 2, 128], bf16, tag="mask")
            nc.gpsimd.dma_start(out=mask, in_=mk)
            b_sb = persist.tile([128, 3, NPAIR], f32, tag="b")
            nc.gpsimd.dma_start(out=b_sb, in_=bqkv)
            ident = persist.tile([128, 64], bf16, tag="ident")
            nc.gpsimd.dma_start(out=ident, in_=idm)

            # ---- weights + x on the sync queue, in consumption order.
            # The DMA engines drain FIFO, so issue order = priority:
            # wq, xq(g0), wk, xk(g0), wv, xv(g0), wo, then remaining x.
            w_sb = {}
            for t_i in range(3):
                w_sb[t_i] = persist.tile(
                    [128, NPAIR, NDC, 128], bf16, tag=f"w{t_i}", name=f"w{t_i}"
                )
            wo_sb = persist.tile([128, NPAIR, D], bf16, tag="wo")
            xall = persist.tile([128, 3, NDC, S], bf16, tag="xall")
            g0 = slice(0, SG)
            wx = {0: (wq, xq), 1: (wk, xk), 2: (wv, xv)}
            # V first: the V->vN dma-transposes gate the first PV matmuls
            # (group 0 is all-diagonal), so V's weights/x/proj come first.
            # Halved loads let the first chunk-matmuls start sooner.
            T_ORDER = (2, 0, 1)
            hc = NDC // 2
            for t_i in T_ORDER:
                wd, xd = wx[t_i]
                for h in range(2):
                    cs = slice(h * hc, (h + 1) * hc)
                    nc.sync.dma_start(out=w_sb[t_i][:, :, cs, :], in_=wd[:, :, cs, :])
                    nc.sync.dma_start(
                        out=xall[:, t_i, cs, g0], in_=xd[:, cs, g0]
                    )
            nc.sync.dma_start(out=wo_sb, in_=wo)

            qT = persist.tile([128, NPAIR, S], bf16, tag="qT")
            kT = persist.tile([128, NPAIR, S], bf16, tag="kT")
            vN = persist.tile([128, HPC, NKT, 65], bf16, tag="vN")
            ctxn = persist.tile([128, NPAIR, S], bf16, tag="ctxn")

            # ones column of V-natural (softmax denominator trick)
            nc.vector.memset(vN[:, :, :, 64:65], 1.0)


            # ---- outproj + proj emitted as "pieces" interleaved into
            # attention phases' kc loops, filling PE gaps where the score->
            # exp->PV chain is Activation-paced.
            def emit_op_half(srow, n, ob):
                op = psC.tile([128, SG], f32, tag="mm", name="op")
                for p in range(NPAIR):
                    nc.tensor.matmul(
                        op,
                        lhsT=ctxn[:, p, srow : srow + 128],
                        rhs=wo_sb[:, p, n * SG : (n + 1) * SG],
                        start=(p == 0),
                        stop=(p == NPAIR - 1),
                    )
                nc.vector.tensor_copy(ob[:, n * SG : (n + 1) * SG], op)

            def outproj_pieces(g):
                pieces = []
                for st4 in range(4):
                    srow = (4 * g + st4) * 128
                    obref = {}

                    def piece0(srow=srow, obref=obref):
                        obref["ob"] = out_pool.tile([128, D], bf16, tag="ob", name="ob")
                        emit_op_half(srow, 0, obref["ob"])

                    def piece1(srow=srow, obref=obref):
                        emit_op_half(srow, 1, obref["ob"])
                        nc.sync.dma_start(
                            out=out[srow : srow + 128, :], in_=obref["ob"]
                        )

                    pieces += [piece0, piece1]
                return pieces

            def proj_pieces(g):
                """Piece-chains for group g's Q/K/V projections (order V,Q,K),
                the V->vN transposes, and the g+2 x prefetch."""
                gs_ = slice(g * SG, (g + 1) * SG)
                pieces = []
                state = {}

                for t_i in T_ORDER:
                    for p_ in range(NPAIR):
                        key = (t_i, p_)

                        def chain_start(t_i=t_i, p_=p_, key=key, gs_=gs_):
                            state[key] = psC.tile([128, SG], f32, tag="mm", name="pp")
                            nc.tensor.matmul(
                                state[key],
                                lhsT=w_sb[t_i][:, p_, 0, :],
                                rhs=xall[:, t_i, 0, gs_],
                                start=True,
                                stop=False,
                            )

                        pieces.append(chain_start)
                        for c in range(1, NDC):

                            def chain_step(c=c, t_i=t_i, p_=p_, key=key, gs_=gs_):
                                nc.tensor.matmul(
                                    state[key],
                                    lhsT=w_sb[t_i][:, p_, c, :],
                                    rhs=xall[:, t_i, c, gs_],
                                    start=False,
                                    stop=(c == NDC - 1),
                                )

                            pieces.append(chain_step)

                        def chain_finish(t_i=t_i, p_=p_, key=key, gs_=gs_):
                            if t_i == 2 and "vtg" not in state:
                                state["vtg"] = vt_pool.tile(
                                    [128, NPAIR, SG], bf16, tag="vtg", name="vtg"
                                )
                            dst = (
                                qT[:, p_, gs_]
                                if t_i == 0
                                else (
                                    kT[:, p_, gs_]
                                    if t_i == 1
                                    else state["vtg"][:, p_, :]
                                )
                            )
                            nc.vector.tensor_scalar_add(
                                out=dst,
                                in0=state[key],
                                scalar1=b_sb[:, t_i, p_ : p_ + 1],
                            )

                        pieces.append(chain_finish)
                    if t_i == 2:
                        # V -> natural layout: 4 PE transposes into one PSUM
                        # tile, one wide copy out (the DMA-XBAR transpose
                        # corrupts data on HW)
                        for p_ in range(NPAIR):
                            for h_s in range(2):

                                def vt_piece(p_=p_, h_s=h_s, g=g):
                                    hp = slice(h_s * 64, (h_s + 1) * 64)
                                    tp = psC.tile(
                                        [128, SG], bf16, tag="mm", name="tp"
                                    )
                                    for k4 in range(4):
                                        nc.tensor.transpose(
                                            tp[:, k4 * 64 : (k4 + 1) * 64],
                                            in_=state["vtg"][
                                                hp, p_, k4 * 128 : (k4 + 1) * 128
                                            ],
                                            identity=ident[hp, :],
                                        )
                                    nc.vector.tensor_copy(
                                        vN[:, 2 * p_ + h_s, 4 * g : 4 * (g + 1), 0:64],
                                        tp[:, 0:256].rearrange(
                                            "p (a b) -> p a b", a=4
                                        ),
                                    )

                                pieces.append(vt_piece)

                # prefetch x two groups ahead (keeps DMA FIFO in
                # consumption order while hiding transfer latency)
                pre = [1, 2] if g == 0 else ([g + 2] if g + 2 < NSG else [])
                for gp in pre:

                    def x_piece(gp=gp):
                        ns_ = slice(gp * SG, (gp + 1) * SG)
                        for t_i in T_ORDER:
                            nc.sync.dma_start(
                                out=xall[:, t_i, :, ns_], in_=wx[t_i][1][:, :, ns_]
                            )

                    pieces.append(x_piece)
                return pieces

            pending = []

            def emit_attn(g):
                """Attention for q-group g, popping `pending` pieces into the
                kc-loop slots to fill PE gaps."""
                gs = slice(g * SG, (g + 1) * SG)
                nkc = (4 * g + 4) if causal else NKT
                total_slots = NPAIR * nkc
                slot = [0]
                len0 = len(pending)
                popped = [0]

                def pop_pieces():
                    # even pacing: spread the filler pieces across the whole
                    # phase so late (exp-paced) kc-steps still get PE work
                    slot[0] += 1
                    target = -(-len0 * slot[0] // total_slots)
                    while pending and popped[0] < target:
                        pending.pop(0)()
                        popped[0] += 1

                for p in range(NPAIR):
                    ctx2 = [
                        psB.tile([65, SG], f32, tag="ctx", name="ctx")
                        for _ in range(2)
                    ]

                    def emit_pv(kc, tp_i, pt2, ctx2=ctx2, p=p, nkc=nkc):
                        pvoff = tp_i * 128 if (causal and tp_i > 0) else 0
                        for h_s in range(2):
                            nc.tensor.matmul(
                                ctx2[h_s][:, pvoff:SG],
                                lhsT=vN[:, 2 * p + h_s, kc, :],
                                rhs=pt2[:, h_s * SG + pvoff : (h_s + 1) * SG],
                                start=(kc == 0),
                                stop=(kc == nkc - 1),
                            )

                    # software-pipelined: sc/exp(kc) emitted before pv(kc-1)
                    prev = None
                    for kc in range(nkc):
                        tp_i = kc - 4 * g
                        diag = causal and tp_i >= 0
                        off = tp_i * 128 if diag else 0
                        sc2 = psA.tile([128, 2 * SG], f32, tag="sc2", name="sc2")
                        for h_s in range(2):
                            hp = slice(h_s * 64, (h_s + 1) * 64)
                            nc.tensor.matmul(
                                sc2[:, h_s * SG + off : (h_s + 1) * SG],
                                lhsT=kT[hp, p, kc * 128 : (kc + 1) * 128],
                                rhs=qT[hp, p, g * SG + off : (g + 1) * SG],
                                start=True,
                                stop=True,
                            )
                        pt2 = pt_pool.tile([128, 2 * SG], bf16, tag="pt", name="pt2")
                        if off == 0:
                            nc.scalar.activation(pt2, sc2, EXP, scale=0.125)
                        else:
                            # one strided activation covering both heads'
                            # [off:SG] columns
                            scv = sc2.rearrange("p (h q) -> p h q", h=2)
                            ptv = pt2.rearrange("p (h q) -> p h q", h=2)
                            nc.scalar.activation(
                                ptv[:, :, off:SG],
                                scv[:, :, off:SG],
                                EXP,
                                scale=0.125,
                            )
                        if diag:
                            d0 = off
                            ptd = pt2.rearrange("p (h q) -> p h q", h=2)[
                                :, :, d0 : d0 + 128
                            ]
                            nc.vector.tensor_mul(ptd, ptd, mask)
                        # ready filler pieces go BEFORE the exp-dependent PV:
                        # the PE parks at most 4 stalled instructions, so
                        # anything emitted after a stalled PV can't bypass it.
                        if prev is not None:
                            pop_pieces()
                            emit_pv(*prev)
                        prev = (kc, tp_i, pt2)
                    pop_pieces()
                    emit_pv(*prev)

                    # ---- per-pair normalize: reciprocal of each head's
                    # denominator row (into a partition-0 tile), gpsimd
                    # partition-broadcast, one multiply per head ----
                    for h_s in range(2):
                        inv = st_pool.tile([1, SG], f32, tag="inv", name="inv")
                        nc.vector.reciprocal(inv, ctx2[h_s][64:65, :])
                        rb = st_pool.tile([64, SG], f32, tag="rb", name="rb")
                        nc.gpsimd.partition_broadcast(rb, inv)
                        hp = slice(h_s * 64, (h_s + 1) * 64)
                        nc.vector.tensor_mul(
                            ctxn[hp, p, gs], ctx2[h_s][0:64, :], rb
                        )

            def run_all(pieces):
                for piece in pieces:
                    piece()
                pieces.clear()

            if causal:
                # outproj pieces are deferred to the LAST attention phases,
                # which otherwise run out of PE filler (attn(3) is the
                # longest phase and exp-paced): op(0)->attn(2),
                # op(1),op(2)->attn(3), op(3)->end flush.
                op_target = {0: 2, 1: 3, 2: 3, 3: None}
                deferred = {g: [] for g in range(NSG)}
                for g in range(NSG):
                    if g == 0:
                        run_all(proj_pieces(0))
                    if g + 1 < NSG:
                        pending.extend(proj_pieces(g + 1))
                    pending.extend(deferred[g])
                    deferred[g] = []
                    emit_attn(g)
                    run_all(pending)
                    tgt = op_target[g]
                    if tgt is None:
                        pending.extend(outproj_pieces(g))
                    else:
                        deferred[tgt].extend(outproj_pieces(g))
                run_all(pending)
            else:
                for g in range(NSG):
                    run_all(proj_pieces(g))
                for g in range(NSG):
                    emit_attn(g)
                    run_all(pending)
                    pending.extend(outproj_pieces(g))
                run_all(pending)

            if DBG:
                nc.sync.dma_start(out=dbg_qT, in_=qT)
                nc.sync.dma_start(out=dbg_kT, in_=kT)
                nc.sync.dma_start(out=dbg_vN, in_=vN)
                nc.sync.dma_start(out=dbg_ctxn, in_=ctxn)

    nc.compile()
    return nc


def _core_inputs(query, key, value, Wq, bq, Wk, bk, Wv, bv, Wo, core):
    b = core // (NCORES // B)
    h0 = (core % (NCORES // B)) * HPC
    f32 = np.float32
    bf16 = np.dtype("bfloat16") if hasattr(np, "bfloat16") else None

    def to_bf16(a):
        import ml_dtypes

        return np.asarray(a, dtype=ml_dtypes.bfloat16)

    def packx(x):
        # x [S, D] -> [128, NDC, S] bf16: out[p, c, s] = x[s, c*128+p]
        xt = np.ascontiguousarray(x.T, dtype=f32).reshape(NDC, 128, S)
        return to_bf16(np.ascontiguousarray(xt.transpose(1, 0, 2)))

    def packw(W):
        # [H, D, DH] -> [128, NPAIR, NDC, 128]: per-pair [D, 128] stacks,
        # then w[p_, pr, c, e] = Wpair[pr][c*128+p_, e]
        pairs = np.stack(
            [
                np.concatenate([W[h0 + 2 * p], W[h0 + 2 * p + 1]], axis=1)
                for p in range(NPAIR)
            ]
        )  # [NPAIR, D, 128]
        w4 = pairs.reshape(NPAIR, NDC, 128, 128).transpose(2, 0, 1, 3)
        return to_bf16(np.ascontiguousarray(w4))

    def packb(bias):
        # [H, DH] -> [128, 3-slot column] handled by caller; here one [128, NPAIR]
        return np.stack(
            [
                np.concatenate([bias[h0 + 2 * p], bias[h0 + 2 * p + 1]])
                for p in range(NPAIR)
            ],
            axis=1,
        ).astype(f32)  # [128, NPAIR]

    # wo: [NPAIR, 128, D] -> [128, NPAIR, D]
    wo_p = np.stack(
        [Wo[(h0 + 2 * p) * DH : (h0 + 2 * p + 2) * DH] for p in range(NPAIR)]
    ).transpose(1, 0, 2)

    jj, ii = np.meshgrid(np.arange(128), np.arange(128), indexing="ij")
    bq3 = np.stack([packb(bq), packb(bk), packb(bv)], axis=1)  # [128, 3, NPAIR]
    return {
        "mk": to_bf16(np.stack([(jj <= ii).astype(f32)] * 2, axis=1)),
        "idm": to_bf16(np.concatenate([np.eye(64, dtype=f32)] * 2, axis=0)),
        "xq": packx(query[b]),
        "xk": packx(key[b]),
        "xv": packx(value[b]),
        "wq": packw(Wq),
        "wk": packw(Wk),
        "wv": packw(Wv),
        "wo": to_bf16(np.ascontiguousarray(wo_p)),
        "bqkv": np.ascontiguousarray(bq3),
    }


LAST_RESULTS = None


def kernel(query, key, value, Wq, bq, Wk, bk, Wv, bv, Wo, bo, look_ahead_mask):
    global LAST_RESULTS
    from concourse.bass_utils import run_bass_kernel_spmd

    query = np.asarray(query, dtype=np.float32)
    key = np.asarray(key, dtype=np.float32)
    value = np.asarray(value, dtype=np.float32)
    Wq, Wk, Wv = (np.asarray(a, dtype=np.float32) for a in (Wq, Wk, Wv))
    bq, bk, bv = (np.asarray(a, dtype=np.float32) for a in (bq, bk, bv))
    Wo = np.asarray(Wo, dtype=np.float32)
    bo = np.asarray(bo, dtype=np.float32)
    causal = bool(np.asarray(look_ahead_mask).item())

    if causal not in _BUILD_CACHE:
        _BUILD_CACHE[causal] = _build(causal)
    nc = _BUILD_CACHE[causal]

    in_maps = [
        _core_inputs(query, key, value, Wq, bq, Wk, bk, Wv, bv, Wo, c)
        for c in range(NCORES)
    ]
    res = run_bass_kernel_spmd(nc, in_maps, core_ids=list(range(NCORES)))
    LAST_RESULTS = res

    gpb = NCORES // B
    out = np.stack(
        [
            np.sum(
                [
                    np.asarray(res.results[b * gpb + i]["out"], dtype=np.float32)
                    for i in range(gpb)
                ],
                axis=0,
            )
            for b in range(B)
        ]
    )
    return (out + bo[None, None, :]).astype(np.float32)


# revision 75
# speedup vs baseline: 1.0033x; 1.0033x over previous
"""Trainium2 Bass kernel for per-head-projection MultiHeadAttention (v3).

Contract: kernel(**inputs) takes the FULL unsharded inputs (as produced by
reference.setup_inputs()) and returns the FULL [B, S, D] output.

Sharding (tensor-parallel over heads x data-parallel over batch):
  - 8 cores; cores 0-3 handle batch 0, cores 4-7 handle batch 1.
  - Each core owns 4 heads (two "head pairs"). It computes Q/K/V projections
    for those heads, causal attention, and a partial output projection
    (ctx @ Wo rows for its heads). The host sums the 4 partials per batch
    (the output linear is linear over head blocks) and adds bo.

v3 (vs v2): bf16 activations/weights on device (fp32 PSUM accumulation);
host pre-packs DRAM tensors in SBUF layout so every load is one large DMA
(DMA-issue overhead: 565ns SP.SEQ + 625ns shared HWDGE + 900ns completion
semaphore each); the softmax denominator is broadcast across partitions
with gpsimd partition_broadcast instead of a DRAM bounce. Work is spread
across engines: exp on ACT, PSUM->SBUF moves/reciprocals on DVE,
diag-mask muls + broadcasts on gpsimd (which cannot access PSUM).

The attention inner loop is Activation(exp)-paced, so all projection and
output-projection matmuls are emitted as "pieces" interleaved into the
attention kc-loops (evenly paced across each phase, emitted BEFORE the
exp-dependent PV so the PE's 4-deep stall window never blocks them);
outproj pieces are deferred to the last, longest attention phases.
Scores are computed transposed ([keys, queries]) so softmax needs no
transposes; the denominator comes from a ones-column in V-natural (built
with PE transposes; the DMA-XBAR transpose corrupts data on HW); exp runs
fused over both heads ([128,1024] PSUM) with the 1/sqrt(DH) scale folded
in. Sharded tensor-parallel over heads x data-parallel over batch as v2.
"""

import sys

sys.path.insert(0, "/opt/trn_rl_repo")

import numpy as np

B, S, D, H = 2, 2048, 1024, 16
DH = D // H            # 64
NCORES = 8
HPC = H * B // NCORES  # 4 heads per core
NPAIR = HPC // 2       # 2 head pairs per core
SG = 512               # s-group / query-group size
NSG = S // SG          # 4
NKT = S // 128         # 16 key tiles
NDC = D // 128         # 8 contraction chunks

_BUILD_CACHE = {}


def _build(causal: bool):
    """Build + compile the per-core Bass program. Cached per causal flag."""
    import concourse.bass as bass
    import concourse.bacc as bacc
    import concourse.tile as tile
    from concourse import mybir

    f32 = mybir.dt.float32
    f32r = mybir.dt.float32r
    bf16 = mybir.dt.bfloat16
    EXP = mybir.ActivationFunctionType.Exp

    nc = bacc.Bacc("TRN2", target_bir_lowering=False, debug=False)

    # DRAM tensors, already in SBUF layout (host packs):
    #   x*  [128, NDC, S]        bf16: x*[p, c, s] = x[s, c*128+p] (x^T rows)
    #   w*  [128, NPAIR, NDC, 128] bf16 per-pair weight stacks
    #   wo  [128, NPAIR, D]      bf16
    #   bqkv [128, 3, NPAIR]     f32 biases
    #   mk  [128, 128]           bf16 lower-triangular ones
    #   on2 [2, 128]             f32r ones2[0,0:64]=1, ones2[1,64:128]=1
    xq = nc.dram_tensor("xq", [128, NDC, S], bf16, kind="ExternalInput").ap()
    xk = nc.dram_tensor("xk", [128, NDC, S], bf16, kind="ExternalInput").ap()
    xv = nc.dram_tensor("xv", [128, NDC, S], bf16, kind="ExternalInput").ap()
    wq = nc.dram_tensor("wq", [128, NPAIR, NDC, 128], bf16, kind="ExternalInput").ap()
    wk = nc.dram_tensor("wk", [128, NPAIR, NDC, 128], bf16, kind="ExternalInput").ap()
    wv = nc.dram_tensor("wv", [128, NPAIR, NDC, 128], bf16, kind="ExternalInput").ap()
    wo = nc.dram_tensor("wo", [128, NPAIR, D], bf16, kind="ExternalInput").ap()
    bqkv = nc.dram_tensor("bqkv", [128, 3, NPAIR], f32, kind="ExternalInput").ap()
    mk = nc.dram_tensor("mk", [128, 2, 128], bf16, kind="ExternalInput").ap()
    idm = nc.dram_tensor("idm", [128, 64], bf16, kind="ExternalInput").ap()
    out = nc.dram_tensor("out", [S, D], bf16, kind="ExternalOutput").ap()
    import os

    DBG = bool(os.environ.get("KERNEL_DEBUG"))
    if DBG:
        dbg_qT = nc.dram_tensor("dbg_qT", [128, NPAIR, S], bf16, kind="ExternalOutput").ap()
        dbg_kT = nc.dram_tensor("dbg_kT", [128, NPAIR, S], bf16, kind="ExternalOutput").ap()
        dbg_vN = nc.dram_tensor(
            "dbg_vN", [128, HPC, NKT, 65], bf16, kind="ExternalOutput"
        ).ap()
        dbg_ctxn = nc.dram_tensor(
            "dbg_ctxn", [128, NPAIR, S], bf16, kind="ExternalOutput"
        ).ap()

    with tile.TileContext(nc) as tc:
        with (
            tc.tile_pool(name="persist", bufs=1) as persist,
            tc.tile_pool(name="vtgs", bufs=3) as vt_pool,
            tc.tile_pool(name="pts", bufs=8) as pt_pool,
            tc.tile_pool(name="invs", bufs=6) as st_pool,
            tc.tile_pool(name="outs", bufs=8) as out_pool,
            tc.tile_pool(name="psma", bufs=2, space="PSUM") as psA,
            tc.tile_pool(name="psmb", bufs=2, space="PSUM") as psB,
            tc.tile_pool(name="psmc", bufs=2, space="PSUM") as psC,
        ):
            # ---- consts (gpsimd/SWDGE queue; tiny, never transfer-critical)
            mask = persist.tile([128, 2, 128], bf16, tag="mask")
            nc.gpsimd.dma_start(out=mask, in_=mk)
            b_sb = persist.tile([128, 3, NPAIR], f32, tag="b")
            nc.gpsimd.dma_start(out=b_sb, in_=bqkv)
            ident = persist.tile([128, 64], bf16, tag="ident")
            nc.gpsimd.dma_start(out=ident, in_=idm)

            # ---- weights + x on the sync queue, in consumption order.
            # The DMA engines drain FIFO, so issue order = priority:
            # wq, xq(g0), wk, xk(g0), wv, xv(g0), wo, then remaining x.
            w_sb = {}
            for t_i in range(3):
                w_sb[t_i] = persist.tile(
                    [128, NPAIR, NDC, 128], bf16, tag=f"w{t_i}", name=f"w{t_i}"
                )
            wo_sb = persist.tile([128, NPAIR, D], bf16, tag="wo")
            xall = persist.tile([128, 3, NDC, S], bf16, tag="xall")
            g0 = slice(0, SG)
            wx = {0: (wq, xq), 1: (wk, xk), 2: (wv, xv)}
            # V first: the V->vN dma-transposes gate the first PV matmuls
            # (group 0 is all-diagonal), so V's weights/x/proj come first.
            # Halved loads let the first chunk-matmuls start sooner.
            T_ORDER = (2, 0, 1)
            hc = NDC // 2
            for t_i in T_ORDER:
                wd, xd = wx[t_i]
                for h in range(2):
                    cs = slice(h * hc, (h + 1) * hc)
                    nc.sync.dma_start(out=w_sb[t_i][:, :, cs, :], in_=wd[:, :, cs, :])
                    nc.sync.dma_start(
                        out=xall[:, t_i, cs, g0], in_=xd[:, cs, g0]
                    )
            # wo is issued after x(g1) below: it isn't consumed until
            # outproj(0) inside attn(1), and here it would delay x(g1) in
            # the DMA FIFO, stalling proj(1) pieces inside attn(0)

            qT = persist.tile([128, NPAIR, S], bf16, tag="qT")
            kT = persist.tile([128, NPAIR, S], bf16, tag="kT")
            vN = persist.tile([128, HPC, NKT, 65], bf16, tag="vN")
            ctxn = persist.tile([128, NPAIR, S], bf16, tag="ctxn")

            # ones column of V-natural (softmax denominator trick)
            nc.vector.memset(vN[:, :, :, 64:65], 1.0)


            # ---- outproj + proj emitted as "pieces" interleaved into
            # attention phases' kc loops, filling PE gaps where the score->
            # exp->PV chain is Activation-paced.
            def emit_op_half(srow, n, ob):
                op = psC.tile([128, SG], f32, tag="mm", name="op")
                for p in range(NPAIR):
                    nc.tensor.matmul(
                        op,
                        lhsT=ctxn[:, p, srow : srow + 128],
                        rhs=wo_sb[:, p, n * SG : (n + 1) * SG],
                        start=(p == 0),
                        stop=(p == NPAIR - 1),
                    )
                nc.vector.tensor_copy(ob[:, n * SG : (n + 1) * SG], op)

            def outproj_pieces(g):
                pieces = []
                for st4 in range(4):
                    srow = (4 * g + st4) * 128
                    obref = {}

                    def piece0(srow=srow, obref=obref):
                        obref["ob"] = out_pool.tile([128, D], bf16, tag="ob", name="ob")
                        emit_op_half(srow, 0, obref["ob"])

                    def piece1(srow=srow, obref=obref):
                        emit_op_half(srow, 1, obref["ob"])
                        nc.sync.dma_start(
                            out=out[srow : srow + 128, :], in_=obref["ob"]
                        )

                    pieces += [piece0, piece1]
                return pieces

            def proj_pieces(g):
                """Piece-chains for group g's Q/K/V projections (order V,Q,K),
                the V->vN transposes, and the g+2 x prefetch."""
                gs_ = slice(g * SG, (g + 1) * SG)
                pieces = []
                state = {}

                for t_i in T_ORDER:
                    for p_ in range(NPAIR):
                        key = (t_i, p_)

                        def chain_start(t_i=t_i, p_=p_, key=key, gs_=gs_):
                            state[key] = psC.tile([128, SG], f32, tag="mm", name="pp")
                            nc.tensor.matmul(
                                state[key],
                                lhsT=w_sb[t_i][:, p_, 0, :],
                                rhs=xall[:, t_i, 0, gs_],
                                start=True,
                                stop=False,
                            )

                        pieces.append(chain_start)
                        for c in range(1, NDC):

                            def chain_step(c=c, t_i=t_i, p_=p_, key=key, gs_=gs_):
                                nc.tensor.matmul(
                                    state[key],
                                    lhsT=w_sb[t_i][:, p_, c, :],
                                    rhs=xall[:, t_i, c, gs_],
                                    start=False,
                                    stop=(c == NDC - 1),
                                )

                            pieces.append(chain_step)

                        def chain_finish(t_i=t_i, p_=p_, key=key, gs_=gs_):
                            if t_i == 2 and "vtg" not in state:
                                state["vtg"] = vt_pool.tile(
                                    [128, NPAIR, SG], bf16, tag="vtg", name="vtg"
                                )
                            dst = (
                                qT[:, p_, gs_]
                                if t_i == 0
                                else (
                                    kT[:, p_, gs_]
                                    if t_i == 1
                                    else state["vtg"][:, p_, :]
                                )
                            )
                            nc.vector.tensor_scalar_add(
                                out=dst,
                                in0=state[key],
                                scalar1=b_sb[:, t_i, p_ : p_ + 1],
                            )

                        pieces.append(chain_finish)
                    if t_i == 2:
                        # V -> natural layout: 4 PE transposes into one PSUM
                        # tile, one wide copy out (the DMA-XBAR transpose
                        # corrupts data on HW)
                        for p_ in range(NPAIR):
                            for h_s in range(2):

                                def vt_piece(p_=p_, h_s=h_s, g=g):
                                    hp = slice(h_s * 64, (h_s + 1) * 64)
                                    tp = psC.tile(
                                        [128, SG], bf16, tag="mm", name="tp"
                                    )
                                    for k4 in range(4):
                                        nc.tensor.transpose(
                                            tp[:, k4 * 64 : (k4 + 1) * 64],
                                            in_=state["vtg"][
                                                hp, p_, k4 * 128 : (k4 + 1) * 128
                                            ],
                                            identity=ident[hp, :],
                                        )
                                    nc.vector.tensor_copy(
                                        vN[:, 2 * p_ + h_s, 4 * g : 4 * (g + 1), 0:64],
                                        tp[:, 0:256].rearrange(
                                            "p (a b) -> p a b", a=4
                                        ),
                                    )

                                pieces.append(vt_piece)

                # prefetch x two groups ahead (keeps DMA FIFO in
                # consumption order while hiding transfer latency)
                pre = [1, 2] if g == 0 else ([g + 2] if g + 2 < NSG else [])
                for gp in pre:

                    def x_piece(gp=gp):
                        ns_ = slice(gp * SG, (gp + 1) * SG)
                        for t_i in T_ORDER:
                            nc.sync.dma_start(
                                out=xall[:, t_i, :, ns_], in_=wx[t_i][1][:, :, ns_]
                            )

                    pieces.append(x_piece)
                if g == 1:
                    # wo after all x prefetches: first consumed by the
                    # outproj(0) pieces deferred into attn(2) (~47us)
                    pieces.append(
                        lambda: nc.sync.dma_start(out=wo_sb, in_=wo)
                    )
                return pieces

            pending = []

            def emit_attn(g):
                """Attention for q-group g, popping `pending` pieces into the
                kc-loop slots to fill PE gaps."""
                gs = slice(g * SG, (g + 1) * SG)
                nkc = (4 * g + 4) if causal else NKT
                total_slots = NPAIR * nkc
                slot = [0]
                len0 = len(pending)
                popped = [0]

                def pop_pieces():
                    # even pacing: spread the filler pieces across the whole
                    # phase so late (exp-paced) kc-steps still get PE work
                    slot[0] += 1
                    target = -(-len0 * slot[0] // total_slots)
                    while pending and popped[0] < target:
                        pending.pop(0)()
                        popped[0] += 1

                for p in range(NPAIR):
                    ctx2 = [
                        psB.tile([65, SG], f32, tag="ctx", name="ctx")
                        for _ in range(2)
                    ]

                    def emit_pv(kc, tp_i, pt2, ctx2=ctx2, p=p, nkc=nkc):
                        pvoff = tp_i * 128 if (causal and tp_i > 0) else 0
                        for h_s in range(2):
                            nc.tensor.matmul(
                                ctx2[h_s][:, pvoff:SG],
                                lhsT=vN[:, 2 * p + h_s, kc, :],
                                rhs=pt2[:, h_s * SG + pvoff : (h_s + 1) * SG],
                                start=(kc == 0),
                                stop=(kc == nkc - 1),
                            )

                    # software-pipelined: sc/exp(kc) emitted before pv(kc-1)
                    prev = None
                    for kc in range(nkc):
                        tp_i = kc - 4 * g
                        diag = causal and tp_i >= 0
                        off = tp_i * 128 if diag else 0
                        sc2 = psA.tile([128, 2 * SG], f32, tag="sc2", name="sc2")
                        for h_s in range(2):
                            hp = slice(h_s * 64, (h_s + 1) * 64)
                            nc.tensor.matmul(
                                sc2[:, h_s * SG + off : (h_s + 1) * SG],
                                lhsT=kT[hp, p, kc * 128 : (kc + 1) * 128],
                                rhs=qT[hp, p, g * SG + off : (g + 1) * SG],
                                start=True,
                                stop=True,
                            )
                        pt2 = pt_pool.tile([128, 2 * SG], bf16, tag="pt", name="pt2")
                        if off == 0:
                            nc.scalar.activation(pt2, sc2, EXP, scale=0.125)
                        else:
                            # one strided activation covering both heads'
                            # [off:SG] columns
                            scv = sc2.rearrange("p (h q) -> p h q", h=2)
                            ptv = pt2.rearrange("p (h q) -> p h q", h=2)
                            nc.scalar.activation(
                                ptv[:, :, off:SG],
                                scv[:, :, off:SG],
                                EXP,
                                scale=0.125,
                            )
                        if diag:
                            d0 = off
                            ptd = pt2.rearrange("p (h q) -> p h q", h=2)[
                                :, :, d0 : d0 + 128
                            ]
                            nc.vector.tensor_mul(ptd, ptd, mask)
                        # ready filler pieces go BEFORE the exp-dependent PV:
                        # the PE parks at most 4 stalled instructions, so
                        # anything emitted after a stalled PV can't bypass it.
                        if prev is not None:
                            pop_pieces()
                            emit_pv(*prev)
                        prev = (kc, tp_i, pt2)
                    pop_pieces()
                    emit_pv(*prev)

                    # ---- per-pair normalize: reciprocal of each head's
                    # denominator row (into a partition-0 tile), gpsimd
                    # partition-broadcast, one multiply per head ----
                    for h_s in range(2):
                        inv = st_pool.tile([1, SG], f32, tag="inv", name="inv")
                        nc.vector.reciprocal(inv, ctx2[h_s][64:65, :])
                        rb = st_pool.tile([64, SG], f32, tag="rb", name="rb")
                        nc.gpsimd.partition_broadcast(rb, inv)
                        hp = slice(h_s * 64, (h_s + 1) * 64)
                        nc.vector.tensor_mul(
                            ctxn[hp, p, gs], ctx2[h_s][0:64, :], rb
                        )

            def run_all(pieces):
                for piece in pieces:
                    piece()
                pieces.clear()

            if causal:
                # outproj pieces are deferred to the LAST attention phases,
                # which otherwise run out of PE filler (attn(3) is the
                # longest phase and exp-paced): op(0)->attn(2),
                # op(1),op(2)->attn(3), op(3)->end flush.
                op_target = {0: 2, 1: 3, 2: 3, 3: None}
                deferred = {g: [] for g in range(NSG)}
                for g in range(NSG):
                    if g == 0:
                        run_all(proj_pieces(0))
                    if g + 1 < NSG:
                        pending.extend(proj_pieces(g + 1))
                    pending.extend(deferred[g])
                    deferred[g] = []
                    emit_attn(g)
                    run_all(pending)
                    tgt = op_target[g]
                    if tgt is None:
                        pending.extend(outproj_pieces(g))
                    else:
                        deferred[tgt].extend(outproj_pieces(g))
                run_all(pending)
            else:
                for g in range(NSG):
                    run_all(proj_pieces(g))
                for g in range(NSG):
                    emit_attn(g)
                    run_all(pending)
                    pending.extend(outproj_pieces(g))
                run_all(pending)

            if DBG:
                nc.sync.dma_start(out=dbg_qT, in_=qT)
                nc.sync.dma_start(out=dbg_kT, in_=kT)
                nc.sync.dma_start(out=dbg_vN, in_=vN)
                nc.sync.dma_start(out=dbg_ctxn, in_=ctxn)

    nc.compile()
    return nc


def _core_inputs(query, key, value, Wq, bq, Wk, bk, Wv, bv, Wo, core):
    b = core // (NCORES // B)
    h0 = (core % (NCORES // B)) * HPC
    f32 = np.float32
    bf16 = np.dtype("bfloat16") if hasattr(np, "bfloat16") else None

    def to_bf16(a):
        import ml_dtypes

        return np.asarray(a, dtype=ml_dtypes.bfloat16)

    def packx(x):
        # x [S, D] -> [128, NDC, S] bf16: out[p, c, s] = x[s, c*128+p]
        xt = np.ascontiguousarray(x.T, dtype=f32).reshape(NDC, 128, S)
        return to_bf16(np.ascontiguousarray(xt.transpose(1, 0, 2)))

    def packw(W):
        # [H, D, DH] -> [128, NPAIR, NDC, 128]: per-pair [D, 128] stacks,
        # then w[p_, pr, c, e] = Wpair[pr][c*128+p_, e]
        pairs = np.stack(
            [
                np.concatenate([W[h0 + 2 * p], W[h0 + 2 * p + 1]], axis=1)
                for p in range(NPAIR)
            ]
        )  # [NPAIR, D, 128]
        w4 = pairs.reshape(NPAIR, NDC, 128, 128).transpose(2, 0, 1, 3)
        return to_bf16(np.ascontiguousarray(w4))

    def packb(bias):
        # [H, DH] -> [128, 3-slot column] handled by caller; here one [128, NPAIR]
        return np.stack(
            [
                np.concatenate([bias[h0 + 2 * p], bias[h0 + 2 * p + 1]])
                for p in range(NPAIR)
            ],
            axis=1,
        ).astype(f32)  # [128, NPAIR]

    # wo: [NPAIR, 128, D] -> [128, NPAIR, D]
    wo_p = np.stack(
        [Wo[(h0 + 2 * p) * DH : (h0 + 2 * p + 2) * DH] for p in range(NPAIR)]
    ).transpose(1, 0, 2)

    jj, ii = np.meshgrid(np.arange(128), np.arange(128), indexing="ij")
    bq3 = np.stack([packb(bq), packb(bk), packb(bv)], axis=1)  # [128, 3, NPAIR]
    return {
        "mk": to_bf16(np.stack([(jj <= ii).astype(f32)] * 2, axis=1)),
        "idm": to_bf16(np.concatenate([np.eye(64, dtype=f32)] * 2, axis=0)),
        "xq": packx(query[b]),
        "xk": packx(key[b]),
        "xv": packx(value[b]),
        "wq": packw(Wq),
        "wk": packw(Wk),
        "wv": packw(Wv),
        "wo": to_bf16(np.ascontiguousarray(wo_p)),
        "bqkv": np.ascontiguousarray(bq3),
    }


LAST_RESULTS = None


def kernel(query, key, value, Wq, bq, Wk, bk, Wv, bv, Wo, bo, look_ahead_mask):
    global LAST_RESULTS
    from concourse.bass_utils import run_bass_kernel_spmd

    query = np.asarray(query, dtype=np.float32)
    key = np.asarray(key, dtype=np.float32)
    value = np.asarray(value, dtype=np.float32)
    Wq, Wk, Wv = (np.asarray(a, dtype=np.float32) for a in (Wq, Wk, Wv))
    bq, bk, bv = (np.asarray(a, dtype=np.float32) for a in (bq, bk, bv))
    Wo = np.asarray(Wo, dtype=np.float32)
    bo = np.asarray(bo, dtype=np.float32)
    causal = bool(np.asarray(look_ahead_mask).item())

    if causal not in _BUILD_CACHE:
        _BUILD_CACHE[causal] = _build(causal)
    nc = _BUILD_CACHE[causal]

    in_maps = [
        _core_inputs(query, key, value, Wq, bq, Wk, bk, Wv, bv, Wo, c)
        for c in range(NCORES)
    ]
    res = run_bass_kernel_spmd(nc, in_maps, core_ids=list(range(NCORES)))
    LAST_RESULTS = res

    gpb = NCORES // B
    out = np.stack(
        [
            np.sum(
                [
                    np.asarray(res.results[b * gpb + i]["out"], dtype=np.float32)
                    for i in range(gpb)
                ],
                axis=0,
            )
            for b in range(B)
        ]
    )
    return (out + bo[None, None, :]).astype(np.float32)
